# revision 20
# baseline (speedup 1.0000x reference)
"""CP/PARAFAC bilinear regression kernel for Trainium2 (8 NeuronCores).

Computes y[n] = beta_0 + sum_{i,j} x[n,i,j] * w[i,j],  w = gamma^T @ alpha.

Data-parallel over the batch axis: each of the 8 cores gets 16384 rows of x.

The reduction is HBM-bandwidth-bound (~15 MB/core at fp16). The kernel
keeps the DMA engines streaming continuously and hides all arithmetic
under them by splitting rows across two independent compute paths
(separate SBUF ports, no contention):

- PE path (rows 0..9215): host lays x out feature-planar fp16, pre-blocked
  by DMA group so every load is one fully-contiguous run per partition.
  Per 512-row window the tensor engine runs 4 matmuls (stationary
  [112,128] = w chunk replicated across columns - keeps the systolic array
  dense so the HAM clock gate stays at 2.4 GHz; moving x^T [112,512] fp16)
  accumulating in PSUM [128,512] fp32 (all rows identical). The scalar
  engine folds +beta_0 from PSUM row 0 into an SBUF y row and two stores
  on its own HWDGE ring stream it out without stalling the x-load FIFO.
- DVE path (rows 9216..16383): natural-layout fp16 row tiles; one fused
  scalar_tensor_tensor per 128-row block does multiply + row-sum in a
  single vector pass (accum_out), writing y columns for a contiguous
  final store (row r = 9216 + p*56 + t on partition p).

w never takes a DRAM round-trip on the critical path: a stride-0-broadcast
tensor_tensor builds B[r,(i,j)] = gamma[r,i]*alpha[r,j], and a
ones-stationary matmul sums over r, leaving w replicated across all 128
PSUM partitions (fp32). The DVE path casts that to fp16 directly; the PE
path's stationary tiles take a small off-critical-path DRAM bounce.

Accuracy: fp16 quantization of x (~2.3e-4) and w (~2.1e-4) only; all
accumulation is fp32.
"""

import numpy as np

N_TOTAL = 131072
N_CORES = 8
N_PER_CORE = N_TOTAL // N_CORES  # 16384
NG = 7
NA = 64
RANK = 64
D = NG * NA  # 448
NCH = 4  # feature chunks (PE path)
CK = D // NCH  # 112
P = 128
WIN = 512  # rows per PSUM window

N_PE = 9216  # rows on the tensor-engine path
N_DVE = N_PER_CORE - N_PE  # 7168 rows on the vector-engine path
TD = N_DVE // P  # 56 y columns on the DVE path

PE_GROUPS = [512, 512, 1024, 2048, 2048, 2048, 1024]
DVE_TILES = [8, 8, 8, 8, 8, 8, 8]
assert sum(PE_GROUPS) == N_PE and sum(DVE_TILES) == TD
N_WARMUP_MM = 8

_CACHE = {}


def _build():
    from concourse import bacc, mybir, tile

    f32 = mybir.dt.float32
    f16 = mybir.dt.float16
    bf16 = mybir.dt.bfloat16

    nc = bacc.Bacc("TRN2", target_bir_lowering=False, debug=False)

    xt_d = nc.dram_tensor("xt", [NCH * CK * N_PE], f16, kind="ExternalInput").ap()
    xn_d = nc.dram_tensor("xn", [N_DVE, D], f16, kind="ExternalInput").ap()
    gamma_d = nc.dram_tensor("gamma", [RANK, NG], f32, kind="ExternalInput").ap()
    alpha_d = nc.dram_tensor("alpha", [RANK, NA], f32, kind="ExternalInput").ap()
    beta_d = nc.dram_tensor("beta", [1], f32, kind="ExternalInput").ap()
    y_d = nc.dram_tensor("y", [N_PER_CORE], f32, kind="ExternalOutput").ap()
    w_d = nc.dram_tensor("w_scratch", [D], f32).ap()

    mult = mybir.AluOpType.mult

    with tile.TileContext(nc) as tc:
        with (
            tc.tile_pool(name="const", bufs=1) as cpool,
            tc.tile_pool(name="xpp", bufs=3) as xppool,
            tc.tile_pool(name="xpd", bufs=4) as xdpool,
            tc.tile_pool(name="sc", bufs=2) as scpool,
            tc.tile_pool(name="ps", bufs=8, space="PSUM") as pspool,
        ):
            # ---- w replicated on 128 partitions, no DRAM hop:
            # B[r, i, j] = gamma[r, i] * alpha[r, j]  (stride-0 broadcasts)
            # psum_rep[m, (i,j)] = sum_r 1 * B[r, (i,j)] = w[(i,j)]  for all m
            g_sb = cpool.tile([RANK, NG], f32)
            a_sb = cpool.tile([RANK, NA], f32)
            nc.sync.dma_start(out=g_sb[:], in_=gamma_d[:, :])
            nc.sync.dma_start(out=a_sb[:], in_=alpha_d[:, :])
            ones_sb = cpool.tile([RANK, P], f32)
            nc.gpsimd.memset(ones_sb[:], 1.0)
            ident = cpool.tile([1, 1], f32)
            nc.gpsimd.memset(ident[:], 1.0)
            b_sb = cpool.tile([RANK, NG, NA], f32)
            nc.vector.tensor_tensor(
                out=b_sb[:],
                in0=g_sb[:, :, None].broadcast_to((RANK, NG, NA)),
                in1=a_sb[:, None, :].broadcast_to((RANK, NG, NA)),
                op=mult,
            )
            psum_rep = pspool.tile([P, D], f32, name="psw", tag="psw")
            nc.tensor.matmul(
                psum_rep[:],
                ones_sb[:],
                b_sb.rearrange("r i j -> r (i j)"),
                start=True,
                stop=True,
            )
            w_rep16 = cpool.tile([P, D], f16)
            nc.vector.tensor_copy(out=w_rep16[:], in_=psum_rep[:])

            # ---- PE-path stationary [112, 4, 128]: transpose w onto 112
            # partitions with 4 tiny PE transposes (no DRAM round-trip),
            # then replicate across 128 columns on GpSimd (dense stationary
            # keeps the systolic array busy so the HAM clock stays warm)
            w_flat = cpool.tile([1, D], f32)
            nc.scalar.copy(out=w_flat[:], in_=psum_rep[:1, :])
            psum_t = pspool.tile([CK, NCH], f32, name="psw", tag="psw")
            for c in range(NCH):
                nc.tensor.transpose(
                    psum_t[:, c : c + 1],
                    w_flat[:, c * CK : (c + 1) * CK],
                    ident[:],
                )
            w16s = cpool.tile([CK, NCH], f16)
            nc.scalar.copy(out=w16s[:], in_=psum_t[:])
            w16big = cpool.tile([CK, NCH, P], f16)
            for c in range(NCH):
                nc.gpsimd.tensor_copy(
                    out=w16big[:, c, :],
                    in_=w16s[:, c : c + 1].broadcast_to((CK, P)),
                )

            # ---- PE HAM warmup: dense bf16 matmuls to lift the clock gate
            # (after the w matmul/transposes in the PE queue, before the
            # real stream)
            wj = cpool.tile([P, P], bf16)
            nc.gpsimd.memset(wj[:], 1.0)
            xj = cpool.tile([P, WIN], bf16)
            nc.gpsimd.memset(xj[:], 1.0)
            for _ in range(N_WARMUP_MM):
                pj = pspool.tile([P, WIN], f32, name="psw", tag="psw")
                nc.tensor.matmul(pj[:], wj[:], xj[:], start=True, stop=True)

            beta_sb = cpool.tile([1, 1], f32)
            nc.scalar.dma_start(out=beta_sb[:], in_=beta_d[None, :])
            beta_bc = cpool.tile([P, 1], f32)

            y_row = cpool.tile([1, N_PE], f32)
            y_sb = cpool.tile([P, TD], f32)

            # DVE rows: partition p holds rows N_PE + p*TD + t
            xn_v = xn_d.rearrange("(p t) c -> p t c", p=P)

            # ---- interleaved main loop: DVE tile first each step, then a
            # PE group; all x loads on the sync ring (one contiguous run
            # per partition each)
            pe_row = 0
            pe_off = 0
            dve_col = 0
            y_stored = 0
            sched = ["p", "d", "p", "d", "p", "d", "p", "d", "p", "d", "p", "d", "p", "d"]
            assert sched.count("d") == len(DVE_TILES)
            assert sched.count("p") == len(PE_GROUPS)
            di = pi = 0
            for kind in sched:
                if kind == "d":
                    t_rows = DVE_TILES[di]
                    di += 1
                    xn = xdpool.tile([P, t_rows, D], f16, name="xn", tag="xn")
                    nc.sync.dma_start(
                        out=xn[:], in_=xn_v[:, dve_col : dve_col + t_rows, :]
                    )
                    for k in range(t_rows):
                        sc = scpool.tile([P, D], f16, name="sc", tag="sc")
                        nc.vector.scalar_tensor_tensor(
                            out=sc[:],
                            in0=xn[:, k, :],
                            scalar=1.0,
                            in1=w_rep16[:],
                            op0=mult,
                            op1=mult,
                            accum_out=y_sb[:, dve_col + k : dve_col + k + 1],
                        )
                    dve_col += t_rows

                else:
                    grows = PE_GROUPS[pi]
                    pi += 1
                    xt = xppool.tile([CK, NCH, grows], f16, name="xt", tag="xt")
                    nc.sync.dma_start(
                        out=xt[:],
                        in_=xt_d[pe_off : pe_off + CK * NCH * grows].rearrange(
                            "(k c j) -> k c j", k=CK, c=NCH
                        ),
                    )
                    nwin = grows // WIN
                    for b0 in range(0, nwin, 4):
                        bw = min(4, nwin - b0)
                        psums = [
                            pspool.tile([P, WIN], f32, name="psw", tag="psw")
                            for _ in range(bw)
                        ]
                        for c in range(NCH):
                            for w in range(bw):
                                j0 = (b0 + w) * WIN
                                nc.tensor.matmul(
                                    psums[w][:],
                                    w16big[:, c, :],
                                    xt[:, c, j0 : j0 + WIN],
                                    start=(c == 0),
                                    stop=(c == NCH - 1),
                                )
                        for w in range(bw):
                            j0 = pe_row + (b0 + w) * WIN
                            nc.scalar.add(
                                out=y_row[:, j0 : j0 + WIN],
                                in_=psums[w][:1, :],
                                add=beta_sb[:],
                            )
                    pe_off += CK * NCH * grows
                    pe_row += grows
                    # first y store once the first 4096 rows are folded
                    if pe_row >= 4096 and y_stored == 0:
                        nc.scalar.dma_start(
                            out=y_d[None, :4096], in_=y_row[:, :4096]
                        )
                        y_stored = 4096

            nc.scalar.dma_start(
                out=y_d[None, y_stored:N_PE], in_=y_row[:, y_stored:]
            )

            # ---- DVE epilogue: +beta, contiguous store of rows N_PE..end
            nc.scalar.dma_start(
                out=beta_bc[:], in_=beta_d[None, :].to_broadcast((P, 1))
            )
            nc.vector.tensor_scalar_add(out=y_sb[:], in0=y_sb[:], scalar1=beta_bc[:])
            nc.scalar.dma_start(
                out=y_d[N_PE:].rearrange("(p t) -> p t", p=P), in_=y_sb[:]
            )

    nc.compile()
    return nc


def _prep_x(x):
    """Full x [131072,7,64] f32 -> per-core (group-blocked planar fp16
    flat [4*112*9216], natural fp16 [7168, 448]).

    Planar block for group g (rows g0:g1) is [112 k, 4 c, rows] so each
    device DMA reads one contiguous run per partition."""
    xf = np.asarray(x, dtype=np.float32).reshape(N_TOTAL, D)
    planar, natural = [], []
    for i in range(N_CORES):
        a = xf[i * N_PER_CORE : (i + 1) * N_PER_CORE]
        at = a[:N_PE].T.astype(np.float16).reshape(NCH, CK, N_PE)  # [c, k, j]
        at = at.transpose(1, 0, 2)  # [k, c, j]
        blocks = []
        g0 = 0
        for grows in PE_GROUPS:
            blocks.append(np.ascontiguousarray(at[:, :, g0 : g0 + grows]).reshape(-1))
            g0 += grows
        planar.append(np.concatenate(blocks))
        natural.append(a[N_PE:].astype(np.float16))
    return planar, natural


def _make_in_maps(x, beta_0, gamma, alpha):
    planar, natural = _prep_x(x)
    gamma_np = np.ascontiguousarray(np.asarray(gamma, dtype=np.float32))
    alpha_np = np.ascontiguousarray(np.asarray(alpha, dtype=np.float32))
    beta_np = np.asarray(beta_0, dtype=np.float32).reshape(1)
    return [
        {
            "xt": planar[i],
            "xn": natural[i],
            "gamma": gamma_np,
            "alpha": alpha_np,
            "beta": beta_np,
        }
        for i in range(N_CORES)
    ]


def kernel(x, beta_0, gamma, alpha):
    from concourse.bass_utils import run_bass_kernel_spmd

    if "nc" not in _CACHE:
        _CACHE["nc"] = _build()
    nc = _CACHE["nc"]

    in_maps = _make_in_maps(x, beta_0, gamma, alpha)
    res = run_bass_kernel_spmd(nc, in_maps, list(range(N_CORES)))
    y = np.concatenate([res.results[i]["y"] for i in range(N_CORES)])
    return y.astype(np.float32)


# revision 21
# speedup vs baseline: 1.1157x; 1.1157x over previous
"""CP/PARAFAC bilinear regression kernel for Trainium2 (8 NeuronCores).

Computes y[n] = beta_0 + sum_{i,j} x[n,i,j] * w[i,j],  w = gamma^T @ alpha.

Data-parallel over the batch axis: each of the 8 cores gets 16384 rows of x.

The reduction is HBM-bandwidth-bound (~15 MB/core at fp16). The kernel
keeps the DMA engines streaming continuously and hides all arithmetic
under them by splitting rows across two independent compute paths
(separate SBUF ports, no contention):

- PE path (rows 0..9215): host lays x out feature-planar fp16, pre-blocked
  by DMA group so every load is one fully-contiguous run per partition.
  Per 512-row window the tensor engine runs 4 matmuls (stationary
  [112,128] = w chunk replicated across columns - keeps the systolic array
  dense so the HAM clock gate stays at 2.4 GHz; moving x^T [112,512] fp16)
  accumulating in PSUM [128,512] fp32 (all rows identical). The scalar
  engine folds +beta_0 from PSUM row 0 into an SBUF y row and two stores
  on its own HWDGE ring stream it out without stalling the x-load FIFO.
- DVE path (rows 9216..16383): natural-layout fp16 row tiles; one fused
  scalar_tensor_tensor per 128-row block does multiply + row-sum in a
  single vector pass (accum_out), writing y columns for a contiguous
  final store (row r = 9216 + p*56 + t on partition p).

w never takes a DRAM round-trip on the critical path: a stride-0-broadcast
tensor_tensor builds B[r,(i,j)] = gamma[r,i]*alpha[r,j], and a
ones-stationary matmul sums over r, leaving w replicated across all 128
PSUM partitions (fp32). The DVE path casts that to fp16 directly; the PE
path's stationary tiles take a small off-critical-path DRAM bounce.

Accuracy: fp16 quantization of x (~2.3e-4) and w (~2.1e-4) only; all
accumulation is fp32.
"""

import numpy as np

N_TOTAL = 131072
N_CORES = 8
N_PER_CORE = N_TOTAL // N_CORES  # 16384
NG = 7
NA = 64
RANK = 64
D = NG * NA  # 448
NCH = 4  # feature chunks (PE path)
CK = D // NCH  # 112
P = 128
WIN = 512  # rows per PSUM window

N_PE = 9216  # rows on the tensor-engine path
N_DVE = N_PER_CORE - N_PE  # 7168 rows on the vector-engine path
TD = N_DVE // P  # 56 y columns on the DVE path

PE_GROUPS = [512, 512, 1024, 2048, 2048, 2048, 1024]
DVE_TILES = [8, 8, 8, 8, 8, 8, 8]
assert sum(PE_GROUPS) == N_PE and sum(DVE_TILES) == TD
N_WARMUP_MM = 5

_CACHE = {}


def _build():
    from concourse import bacc, mybir, tile

    f32 = mybir.dt.float32
    f16 = mybir.dt.float16
    bf16 = mybir.dt.bfloat16

    nc = bacc.Bacc("TRN2", target_bir_lowering=False, debug=False)

    xt_d = nc.dram_tensor("xt", [NCH * CK * N_PE], f16, kind="ExternalInput").ap()
    xn_d = nc.dram_tensor("xn", [N_DVE, D], f16, kind="ExternalInput").ap()
    gamma_d = nc.dram_tensor("gamma", [RANK, NG], f32, kind="ExternalInput").ap()
    alpha_d = nc.dram_tensor("alpha", [RANK, NA], f32, kind="ExternalInput").ap()
    beta_d = nc.dram_tensor("beta", [1], f32, kind="ExternalInput").ap()
    y_d = nc.dram_tensor("y", [N_PER_CORE], f32, kind="ExternalOutput").ap()
    w_d = nc.dram_tensor("w_scratch", [D], f32).ap()

    mult = mybir.AluOpType.mult

    with tile.TileContext(nc) as tc:
        with (
            tc.tile_pool(name="const", bufs=1) as cpool,
            tc.tile_pool(name="xpp", bufs=4) as xppool,
            tc.tile_pool(name="xpd", bufs=5) as xdpool,
            tc.tile_pool(name="sc", bufs=2) as scpool,
            tc.tile_pool(name="ps", bufs=8, space="PSUM") as pspool,
        ):
            # ---- PE HAM warmup: dense bf16 matmuls, first in the PE queue
            # (gated only by GpSimd memsets, fills the preamble idle window)
            wj = cpool.tile([P, P], bf16)
            nc.gpsimd.memset(wj[:], 1.0)
            xj = cpool.tile([P, WIN], bf16)
            nc.gpsimd.memset(xj[:], 1.0)
            for _ in range(N_WARMUP_MM):
                pj = pspool.tile([P, WIN], f32, name="psw", tag="psw")
                nc.tensor.matmul(pj[:], wj[:], xj[:], start=True, stop=True)

            # ---- w replicated on 128 partitions, no DRAM hop:
            # B[r, i, j] = gamma[r, i] * alpha[r, j]  (stride-0 broadcasts)
            # psum_rep[m, (i,j)] = sum_r 1 * B[r, (i,j)] = w[(i,j)]  for all m
            g_sb = cpool.tile([RANK, NG], f32)
            a_sb = cpool.tile([RANK, NA], f32)
            nc.sync.dma_start(out=g_sb[:], in_=gamma_d[:, :])
            nc.sync.dma_start(out=a_sb[:], in_=alpha_d[:, :])
            ones_sb = cpool.tile([RANK, P], f32)
            nc.gpsimd.memset(ones_sb[:], 1.0)
            ident = cpool.tile([1, 1], f32)
            nc.gpsimd.memset(ident[:], 1.0)
            b_sb = cpool.tile([RANK, NG, NA], f32)
            nc.vector.tensor_tensor(
                out=b_sb[:],
                in0=g_sb[:, :, None].broadcast_to((RANK, NG, NA)),
                in1=a_sb[:, None, :].broadcast_to((RANK, NG, NA)),
                op=mult,
            )
            psum_rep = pspool.tile([P, D], f32, name="psw", tag="psw")
            nc.tensor.matmul(
                psum_rep[:],
                ones_sb[:],
                b_sb.rearrange("r i j -> r (i j)"),
                start=True,
                stop=True,
            )
            w_rep16 = cpool.tile([P, D], f16)
            nc.vector.tensor_copy(out=w_rep16[:], in_=psum_rep[:])

            # ---- PE-path stationary [112, 4, 128]: transpose w onto 112
            # partitions with 4 tiny PE transposes (no DRAM round-trip),
            # then replicate across 128 columns on GpSimd (dense stationary
            # keeps the systolic array busy so the HAM clock stays warm)
            w_flat = cpool.tile([1, D], f32)
            nc.scalar.copy(out=w_flat[:], in_=psum_rep[:1, :])
            psum_t = pspool.tile([CK, NCH], f32, name="psw", tag="psw")
            for c in range(NCH):
                nc.tensor.transpose(
                    psum_t[:, c : c + 1],
                    w_flat[:, c * CK : (c + 1) * CK],
                    ident[:],
                )
            w16s = cpool.tile([CK, NCH], f16)
            nc.scalar.copy(out=w16s[:], in_=psum_t[:])
            w16big = cpool.tile([CK, NCH, P], f16)
            for c in range(NCH):
                nc.gpsimd.tensor_copy(
                    out=w16big[:, c, :],
                    in_=w16s[:, c : c + 1].broadcast_to((CK, P)),
                )

            beta_sb = cpool.tile([1, 1], f32)
            nc.scalar.dma_start(out=beta_sb[:], in_=beta_d[None, :])
            beta_bc = cpool.tile([P, 1], f32)

            y_row = cpool.tile([1, N_PE], f32)
            y_sb = cpool.tile([P, TD], f32)

            # DVE rows: partition p holds rows N_PE + p*TD + t
            xn_v = xn_d.rearrange("(p t) c -> p t c", p=P)

            # ---- interleaved main loop: DVE tile first each step, then a
            # PE group; all x loads on the sync ring (one contiguous run
            # per partition each)
            pe_row = 0
            pe_off = 0
            dve_col = 0
            y_stored = 0
            sched = ["p", "d", "p", "d", "p", "d", "p", "d", "p", "d", "p", "d", "p", "d"]
            assert sched.count("d") == len(DVE_TILES)
            assert sched.count("p") == len(PE_GROUPS)
            di = pi = 0
            for kind in sched:
                if kind == "d":
                    t_rows = DVE_TILES[di]
                    di += 1
                    xn = xdpool.tile([P, t_rows, D], f16, name="xn", tag="xn")
                    nc.sync.dma_start(
                        out=xn[:], in_=xn_v[:, dve_col : dve_col + t_rows, :]
                    )
                    for k in range(t_rows):
                        sc = scpool.tile([P, D], f16, name="sc", tag="sc")
                        nc.vector.scalar_tensor_tensor(
                            out=sc[:],
                            in0=xn[:, k, :],
                            scalar=1.0,
                            in1=w_rep16[:],
                            op0=mult,
                            op1=mult,
                            accum_out=y_sb[:, dve_col + k : dve_col + k + 1],
                        )
                    dve_col += t_rows

                else:
                    grows = PE_GROUPS[pi]
                    pi += 1
                    xt = xppool.tile([CK, NCH, grows], f16, name="xt", tag="xt")
                    nc.sync.dma_start(
                        out=xt[:],
                        in_=xt_d[pe_off : pe_off + CK * NCH * grows].rearrange(
                            "(k c j) -> k c j", k=CK, c=NCH
                        ),
                    )
                    nwin = grows // WIN
                    for b0 in range(0, nwin, 4):
                        bw = min(4, nwin - b0)
                        psums = [
                            pspool.tile([P, WIN], f32, name="psw", tag="psw")
                            for _ in range(bw)
                        ]
                        for c in range(NCH):
                            for w in range(bw):
                                j0 = (b0 + w) * WIN
                                nc.tensor.matmul(
                                    psums[w][:],
                                    w16big[:, c, :],
                                    xt[:, c, j0 : j0 + WIN],
                                    start=(c == 0),
                                    stop=(c == NCH - 1),
                                )
                        for w in range(bw):
                            j0 = pe_row + (b0 + w) * WIN
                            nc.scalar.add(
                                out=y_row[:, j0 : j0 + WIN],
                                in_=psums[w][:1, :],
                                add=beta_sb[:],
                            )
                    pe_off += CK * NCH * grows
                    pe_row += grows
                    # first y store once the first 4096 rows are folded
                    if pe_row >= 4096 and y_stored == 0:
                        nc.scalar.dma_start(
                            out=y_d[None, :4096], in_=y_row[:, :4096]
                        )
                        y_stored = 4096

            nc.scalar.dma_start(
                out=y_d[None, y_stored:N_PE], in_=y_row[:, y_stored:]
            )

            # ---- DVE epilogue: +beta, contiguous store of rows N_PE..end
            nc.scalar.dma_start(
                out=beta_bc[:], in_=beta_d[None, :].to_broadcast((P, 1))
            )
            nc.vector.tensor_scalar_add(out=y_sb[:], in0=y_sb[:], scalar1=beta_bc[:])
            nc.scalar.dma_start(
                out=y_d[N_PE:].rearrange("(p t) -> p t", p=P), in_=y_sb[:]
            )

    nc.compile()
    return nc


def _prep_x(x):
    """Full x [131072,7,64] f32 -> per-core (group-blocked planar fp16
    flat [4*112*9216], natural fp16 [7168, 448]).

    Planar block for group g (rows g0:g1) is [112 k, 4 c, rows] so each
    device DMA reads one contiguous run per partition."""
    xf = np.asarray(x, dtype=np.float32).reshape(N_TOTAL, D)
    planar, natural = [], []
    for i in range(N_CORES):
        a = xf[i * N_PER_CORE : (i + 1) * N_PER_CORE]
        at = a[:N_PE].T.astype(np.float16).reshape(NCH, CK, N_PE)  # [c, k, j]
        at = at.transpose(1, 0, 2)  # [k, c, j]
        blocks = []
        g0 = 0
        for grows in PE_GROUPS:
            blocks.append(np.ascontiguousarray(at[:, :, g0 : g0 + grows]).reshape(-1))
            g0 += grows
        planar.append(np.concatenate(blocks))
        natural.append(a[N_PE:].astype(np.float16))
    return planar, natural


def _make_in_maps(x, beta_0, gamma, alpha):
    planar, natural = _prep_x(x)
    gamma_np = np.ascontiguousarray(np.asarray(gamma, dtype=np.float32))
    alpha_np = np.ascontiguousarray(np.asarray(alpha, dtype=np.float32))
    beta_np = np.asarray(beta_0, dtype=np.float32).reshape(1)
    return [
        {
            "xt": planar[i],
            "xn": natural[i],
            "gamma": gamma_np,
            "alpha": alpha_np,
            "beta": beta_np,
        }
        for i in range(N_CORES)
    ]


def kernel(x, beta_0, gamma, alpha):
    from concourse.bass_utils import run_bass_kernel_spmd

    if "nc" not in _CACHE:
        _CACHE["nc"] = _build()
    nc = _CACHE["nc"]

    in_maps = _make_in_maps(x, beta_0, gamma, alpha)
    res = run_bass_kernel_spmd(nc, in_maps, list(range(N_CORES)))
    y = np.concatenate([res.results[i]["y"] for i in range(N_CORES)])
    return y.astype(np.float32)
